# revision 28
# baseline (speedup 1.0000x reference)
"""Bass/Trainium2 kernel for nn_BipartPool: bipartite attention pooling.

Model (B=64 graphs, N=128 nodes/graph, R=32 aggregator queries/graph,
H=8 heads, HD=64, E=512):
  q = (aggrs @ Wq.T + bq) / sqrt(HD)   -- identical for every graph
  k = x @ Wk.T, v = x @ Wv.T            (per node)
  per graph g, head h: attn = softmax(q_h k_{g,h}^T)
  out_g = concat_h(attn @ v_{g,h}) @ Wo.T + bo

Sharding: data-parallel over graphs, 8 graphs per core x 8 cores.
Replicated weights, no collectives.

Exact algebraic simplifications (host-side constant folding):
  - bk drops out of softmax; bv folds into bo_eff = Wo @ bv + bo.
  - A^T[e, (h,q)] = Wk_h.T q'_hq with the 1/sqrt(HD) scale folded in.

Device pipeline per core (G=8 graphs, S=1024 nodes), transpose-free:
  scoresT_g[node, (h q)] = sum_ec xT_ec_g.T @ aT_ec    (PSUM, per graph)
  exp (ACT) -> denominators via ONES[128,128] matmul, which lands the
  column sums replicated on every PSUM partition -> full-width DVE
  reciprocal -> GpSimd multiply = normalized attnT, node-major.
  V_g = x_g @ Wv.T;  yT_h = matmul(lhsT=v_gh, rhs=attnT slice)
  out = yT.T @ WoT (+ bias during the PSUM->SBUF add) -> DMA out
The PE never transposes anything and runs no fp32 matmuls.  HAM
warm-up: a burst of small matmuls on memset garbage while the first
DMAs are still in flight.
"""

import ml_dtypes
import numpy as np

import concourse.bacc as bacc
import concourse.mybir as mybir
from concourse import tile
from concourse.bass_utils import run_bass_kernel_spmd

F32 = mybir.dt.float32
F16 = mybir.dt.float16
AF = mybir.ActivationFunctionType

B, N, RATIO, H, HD = 64, 128, 32, 8, 64
E = H * HD                 # 512
NCORES = 8
G = B // NCORES            # 8 graphs per core
S = G * N                  # 1024 nodes per core
EC = E // 128              # 4 contraction chunks
HQ = H * RATIO             # 256 (head, query) pairs
NWARM = 36                 # HAM warm-up matmuls

_CACHE = {}
LAST_RESULT = None         # test harness reads exec_time_ns from here


def _emit(nc, tc, d):
    with (
        nc.allow_low_precision(reason="f16 intermediates are intended"),
        tc.tile_pool(name="sb", bufs=1) as sb,
        tc.tile_pool(name="psS", bufs=2, space="PSUM") as psS,
        tc.tile_pool(name="psV", bufs=2, space="PSUM") as psV,
        tc.tile_pool(name="psD", bufs=2, space="PSUM") as psD,
        tc.tile_pool(name="psY", bufs=2, space="PSUM") as psY,
    ):
        # ---- persistent SBUF tensors -------------------------------------
        x_sb = sb.tile([128, 4, EC, 256], F16)      # xT [feat-part, pair, ec, node]
        a_sb = sb.tile([128, EC, HQ], F16)          # A^T [feat-part, ec, (h q)]
        wv_sb = sb.tile([128, EC, E], F16)          # WvT [feat-part, ec, f]
        wo_sb = sb.tile([128, EC, E], F16)          # WoT [hd-part, hp, e]
        bo_sb = sb.tile([1, E], F16)
        ones_sb = sb.tile([128, 128], F16)          # colsum-replicate lhsT
        ex_sb = sb.tile([128, G, HQ], F16)          # exp(scoresT) [node, g, hq]
        rb_sb = sb.tile([128, G, HQ], F32)          # 1/denominator (all parts)
        at_sb = sb.tile([128, G, HQ], F16)          # attnT [node, g, (h q)]
        v_sb = sb.tile([128, G, E], F16)            # V  [node, g, f]
        y_sb = sb.tile([128, EC, 2, 128], F16)      # yT [head-pair d, hp, gg, gq]
        o_sb = sb.tile([128, 2, E], F32)            # output rows

        # ---- memsets first: warm-up data with no DMA dependency ----------
        nc.gpsimd.memset(ones_sb[:], 1.0)

        # ---- DMA in: contiguous 2KB-per-partition transfers --------------
        nc.scalar.dma_start(out=a_sb[:], in_=d["aT"][:])
        for p in range(4):
            nc.sync.dma_start(out=x_sb[:, p, :, :], in_=d["xT"][:, p, :, :])
        nc.gpsimd.dma_start(out=wv_sb[:, 0:2, :], in_=d["wvT"][:, 0:2, :])
        nc.scalar.dma_start(out=wv_sb[:, 2:4, :], in_=d["wvT"][:, 2:4, :])
        nc.gpsimd.dma_start(out=wo_sb[:], in_=d["woT"][:])
        nc.scalar.dma_start(out=bo_sb[:], in_=d["bo"][:])

        # ---- HAM warm-up: garbage matmuls while inputs stream ------------
        wp = psY.tile([64, 128], F32, tag="yp", name="warm")
        for w in range(NWARM):
            nc.tensor.matmul(wp[:], (ones_sb[:, 0:64]), (ones_sb[:]),
                             start=(w == 0), stop=(w == NWARM - 1))

        def scores_pair(p):
            """scoresT for graphs 2p, 2p+1 -> exp -> normalized attnT."""
            sp = psS.tile([128, 2, HQ], F32, tag="sp", name=f"sp{p}")
            for j in range(2):
                g = 2 * p + j
                for ec in range(EC):
                    nc.tensor.matmul(
                        sp[:, j, :],
                        (x_sb[:, g // 2, ec, (g % 2) * 128:(g % 2) * 128 + 128]),
                        (a_sb[:, ec, :]),
                        start=(ec == 0), stop=(ec == EC - 1),
                    )
                nc.scalar.activation(ex_sb[:, g, :], sp[:, j, :], AF.Exp)
                dn = psD.tile([128, HQ], F32, tag="dn", name=f"dn{g}")
                nc.tensor.matmul(dn[:], (ones_sb[:]), (ex_sb[:, g, :]),
                                 start=True, stop=True)
                nc.vector.reciprocal_approx_fast(rb_sb[:, g, :], dn[:])
                nc.vector.tensor_mul(at_sb[:, g, :], ex_sb[:, g, :],
                                     rb_sb[:, g, :])

        def v_proj(g):
            vp = psV.tile([128, E], F32, tag="vp", name=f"vp{g}")
            for ec in range(EC):
                nc.tensor.matmul(
                    vp[:],
                    (x_sb[:, g // 2, ec, (g % 2) * 128:(g % 2) * 128 + 128]),
                    (wv_sb[:, ec, :]),
                    start=(ec == 0), stop=(ec == EC - 1),
                )
            if g % 2 == 0:
                nc.scalar.copy(v_sb[:, g, :], vp[:])
            else:
                nc.vector.tensor_copy(v_sb[:, g, :], vp[:])

        def attn_gg(gg):
            """yT tiles for graph-group gg (graphs 4gg..4gg+3), head pairs
            col-tiled to partitions 0/64 so one [128,128] copy serves 2 heads."""
            for hp in range(EC):
                yp = psY.tile([128, 128], F32, tag="yp", name=f"yp{gg}{hp}")
                for hh in range(2):
                    h = 2 * hp + hh
                    for jg in range(4):
                        g = gg * 4 + jg
                        nc.tensor.matmul(
                            yp[hh * 64:(hh + 1) * 64, jg * 32:(jg + 1) * 32],
                            (v_sb[:, g, h * 64:(h + 1) * 64]),
                            (at_sb[:, g, h * 32:(h + 1) * 32]),
                            start=True, stop=True,
                        )
                if hp % 2 == 0:
                    nc.vector.tensor_copy(y_sb[:, hp, gg, :], yp[:])
                else:
                    nc.scalar.copy(y_sb[:, hp, gg, :], yp[:])

        def out_gg(gg):
            op = psS.tile([128, E], F32, tag="sp", name=f"op{gg}")
            for hp in range(EC):
                nc.tensor.matmul(op[:], (y_sb[:, hp, gg, :]), (wo_sb[:, hp, :]),
                                 start=(hp == 0), stop=False)
            # bias as a rank-1 matmul into the same accumulation group
            nc.tensor.matmul(op[:], (ones_sb[0:1, :]), (bo_sb[:]),
                             start=False, stop=True)
            # split stores across two queues
            q0 = nc.sync if gg == 0 else nc.scalar
            q1 = nc.scalar if gg == 0 else nc.sync
            nc.vector.tensor_copy(o_sb[0:64, gg, :], op[0:64, :])
            q0.dma_start(out=d["out"][gg * 128:gg * 128 + 64, :],
                         in_=o_sb[0:64, gg, :])
            nc.vector.tensor_copy(o_sb[64:128, gg, :], op[64:128, :])
            q1.dma_start(out=d["out"][gg * 128 + 64:(gg + 1) * 128, :],
                         in_=o_sb[64:128, gg, :])

        # ---- pipeline ----------------------------------------------------
        scores_pair(0)
        scores_pair(1)
        v_proj(0); v_proj(1)
        v_proj(2); v_proj(3)
        scores_pair(2)
        v_proj(4); v_proj(5)
        attn_gg(0)
        scores_pair(3)
        v_proj(6); v_proj(7)
        out_gg(0)
        attn_gg(1)
        out_gg(1)


def _build():
    nc = bacc.Bacc("TRN2", target_bir_lowering=False, debug=False,
                   enable_asserts=False)
    d = {}
    d["xT"] = nc.dram_tensor("xT", (128, 4, EC, 256), F16, kind="ExternalInput").ap()
    d["aT"] = nc.dram_tensor("aT", (128, EC, HQ), F16, kind="ExternalInput").ap()
    d["wvT"] = nc.dram_tensor("wvT", (128, EC, E), F16, kind="ExternalInput").ap()
    d["woT"] = nc.dram_tensor("woT", (128, EC, E), F16, kind="ExternalInput").ap()
    d["bo"] = nc.dram_tensor("bo", (1, E), F16, kind="ExternalInput").ap()
    d["out"] = nc.dram_tensor("out", (G * RATIO, E), F32, kind="ExternalOutput").ap()
    with tile.TileContext(nc) as tc:
        _emit(nc, tc, d)
    nc.compile()
    return nc


def host_prep(x, aggrs, in_proj_w, in_proj_b, out_proj_w, out_proj_b):
    """Constant-fold the input-independent weight algebra; shard x."""
    x = np.asarray(x, dtype=np.float32)
    aggrs = np.asarray(aggrs, dtype=np.float32)
    in_proj_w = np.asarray(in_proj_w, dtype=np.float32)
    in_proj_b = np.asarray(in_proj_b, dtype=np.float32)
    out_proj_w = np.asarray(out_proj_w, dtype=np.float32)
    out_proj_b = np.asarray(out_proj_b, dtype=np.float32)

    scale = np.float32(1.0 / np.sqrt(HD))
    wq, wk, wv = in_proj_w[:E], in_proj_w[E:2 * E], in_proj_w[2 * E:]
    bv = in_proj_b[2 * E:]
    q = (aggrs @ wq.T + in_proj_b[:E]) * scale          # q' = scaled queries
    aT = np.empty((E, HQ), dtype=np.float32)            # A^T[e, h*R+r]
    for h in range(H):
        aT[:, h * RATIO:(h + 1) * RATIO] = wk[h * HD:(h + 1) * HD, :].T @ \
            q[:, h * HD:(h + 1) * HD].T

    def chunked(m, ncols):                               # [E, ncols] -> [128, EC, ncols]
        return np.ascontiguousarray(
            m.reshape(EC, 128, ncols).transpose(1, 0, 2)).astype(np.float16)

    shared = {
        "aT": chunked(aT, HQ),
        "wvT": chunked(np.ascontiguousarray(wv.T), E),
        "woT": chunked(np.ascontiguousarray(out_proj_w.T), E),
        "bo": (out_proj_w @ bv + out_proj_b).reshape(1, E).astype(np.float16),
    }
    in_maps = []
    for c in range(NCORES):
        m = dict(shared)
        xc = np.ascontiguousarray(x[c * G:(c + 1) * G].reshape(S, E).T)
        m["xT"] = np.ascontiguousarray(
            xc.reshape(EC, 128, 4, 256).transpose(1, 2, 0, 3)).astype(np.float16)
        in_maps.append(m)
    return in_maps


def kernel(x, batch, aggrs, in_proj_w, in_proj_b, out_proj_w, out_proj_b):
    global LAST_RESULT
    in_maps = host_prep(x, aggrs, in_proj_w, in_proj_b, out_proj_w, out_proj_b)
    if "nc" not in _CACHE:
        _CACHE["nc"] = _build()
    res = run_bass_kernel_spmd(_CACHE["nc"], in_maps, list(range(NCORES)))
    LAST_RESULT = res
    out = np.concatenate([res.results[c]["out"] for c in range(NCORES)], axis=0)
    return out.reshape(B, RATIO, E).astype(np.float32)
